# revision 10
# baseline (speedup 1.0000x reference)
"""Trainium2 Bass kernel for nn_BalRNN (2-layer sparse RNN).

Strategy (8 NeuronCores, data-parallel over batch):
  - Each core owns B_local=4 batch elements; scan over T=64 is local.
  - The sparse recurrent matmul out[b,i] = sum_k vals[i,k] * h[b, idx[i,k]]
    is computed as a GPSIMD ap_gather (per-Q7-core index streams, shared
    across each core's 16 partitions) followed by DVE segmented reduces
    over the K axis.  hh_vals is piecewise-constant in k (two groups), so
    the multiply folds into two scalar constants applied after the reduce.
  - The gather table (full h, replicated to every partition; partition
    lane encodes the batch element) is rebuilt each step with a TensorE
    broadcast matmul (0/1 selection matrices) + ScalarE PSUM->SBUF copy.
  - The input projection xe = x @ w_ih.T runs once on TensorE at setup and
    is staged in DRAM in the per-timestep replicated layout.

Layout per NeuronCore (128 SBUF partitions):
  partition p = 16*c + lane, c = Q7 core 0..7, lane = 4*r + b
  (b = batch 0..3, r = replica 0..3).  Q7 core c computes output rows
  [512c, 512c+512); its 16 lanes hold tables for batches b = lane % 4.
"""

import sys

sys.path.insert(0, "/opt/trn_rl_repo")

import numpy as np

import concourse.bacc as bacc
import concourse.mybir as mybir
import concourse.tile as tile
from concourse.bass_utils import run_bass_kernel_spmd

B, T, IN = 32, 64, 512
L, H, K = 2, 4096, 64
NCORES = 8
BL = B // NCORES          # 4 batches per NeuronCore
NQ7 = 8                   # Q7 cores per NeuronCore
ROWS_PER_CORE = H // NQ7  # 512
CHUNKS = 4                # gather chunks per spmm
ROWS_PER_CHUNK = ROWS_PER_CORE // CHUNKS   # 128
IDX_PER_CHUNK = ROWS_PER_CHUNK * K         # 8192
IDXCOLS = ROWS_PER_CORE * K // 16          # 2048 idx columns per partition
F32 = mybir.dt.float32
I16 = mybir.dt.int16

_CACHE = {}


def _build_program(c_consts, k1):
    """Trace + compile the bass program.

    c_consts: ((c1_l0, c2_l0), (c1_l1, c2_l1)) fp32 python floats
    k1: split point in k (first k1 entries use c1, rest use c2)
    """
    k2 = K - k1
    nc = bacc.Bacc("TRN2", target_bir_lowering=False, debug=False,
                   num_devices=NCORES)

    # ---- DRAM I/O ----
    xT = nc.dram_tensor("xT", [IN, BL * T], F32, kind="ExternalInput").ap()
    wT = nc.dram_tensor("wT", [IN, H], F32, kind="ExternalInput").ap()
    esel = nc.dram_tensor("esel", [NQ7, 128, 128], F32,
                          kind="ExternalInput").ap()
    gidx = nc.dram_tensor("gidx", [L, 128, IDXCOLS], I16,
                          kind="ExternalInput").ap()
    out_seq = nc.dram_tensor("out_seq", [T, 128, ROWS_PER_CORE], F32,
                             kind="ExternalOutput").ap()
    hfin = nc.dram_tensor("hfin", [L, 128, ROWS_PER_CORE], F32,
                          kind="ExternalOutput").ap()
    xe_rep = nc.dram_tensor("xe_rep", [T, 128, ROWS_PER_CORE], F32).ap()

    with tile.TileContext(nc) as tc:
        with (
            tc.tile_pool(name="const", bufs=1) as constp,
            tc.tile_pool(name="ps", bufs=1, space="PSUM") as psp,
        ):
            # ---- load constants ----
            e_sb = constp.tile([128, NQ7 * 128], F32)
            nc.sync.dma_start(e_sb[:].rearrange("p (c m) -> p c m", c=NQ7),
                              esel.rearrange("c p m -> p c m"))
            gidx_sb = []
            for l in range(L):
                g = constp.tile([128, IDXCOLS], I16, tag=f"gidx{l}")
                nc.sync.dma_start(g[:], gidx[l])
                gidx_sb.append(g)

            # ---- xe = x @ w_ih.T  (once, on PE) ----
            setupp_cm = tc.tile_pool(name="setup", bufs=1)
            setupp = setupp_cm.__enter__()
            xT_sb = setupp.tile([128, 4 * BL * T], F32)   # [j, (jb, m)]
            nc.sync.dma_start(
                xT_sb[:].rearrange("p (jb m) -> p jb m", jb=4),
                xT.rearrange("(jb p) m -> p jb m", p=128))
            wT_sb = setupp.tile([128, 4 * H], F32)        # [j, (jb, i)]
            nc.sync.dma_start(
                wT_sb[:].rearrange("p (jb i) -> p jb i", jb=4),
                wT.rearrange("(jb p) i -> p jb i", p=128))

            M = BL * T
            n_mb = M // 128
            for mb in range(n_mb):
                ps = psp.tile([128, H], F32, tag="tps")
                for c in range(NQ7):
                    for jb in range(4):
                        nc.tensor.matmul(
                            ps[:, 512 * c:512 * c + 512],
                            xT_sb[:, M * jb + 128 * mb:
                                  M * jb + 128 * mb + 128],
                            wT_sb[:, H * jb + 512 * c:H * jb + 512 * c + 512],
                            start=(jb == 0), stop=(jb == 3))
                xe_sb = setupp.tile([128, H], F32, tag=f"xe_sb{mb}")
                nc.scalar.copy(xe_sb[:], ps[:])
                # scatter into DRAM replicated layout:
                # xe_rep[t, 16c+4r+b, j'] = xe[(b,t), 512c + j']
                for b in range(BL):
                    m0 = b * T
                    if m0 // 128 != mb:
                        continue
                    off = m0 - 128 * mb
                    src = xe_sb[off:off + T, :]
                    for r in range(4):
                        lane = 4 * r + b
                        dst = xe_rep.rearrange(
                            "t (c l) j -> t c l j", l=16)[:, :, lane, :]
                        nc.sync.dma_start(dst, src.rearrange(
                            "t (c j) -> t c j", c=NQ7))

            setupp_cm.__exit__(None, None, None)

            # ---- recurrence ----
            rec_pool_cms = (
                tc.tile_pool(name="tables", bufs=1),
                tc.tile_pool(name="gbuf", bufs=3),
                tc.tile_pool(name="state", bufs=3),
                tc.tile_pool(name="small", bufs=2),
                tc.tile_pool(name="xt", bufs=3),
            )
            tablep, gpool, statep, smallp, xtp = \
                (cm.__enter__() for cm in rec_pool_cms)
            cc = c_consts

            def build_table(src_ap, tag):
                """table[p, :H]: partition p=(c,lane) <- h[b(lane), :]."""
                ps = psp.tile([128, H], F32, tag="tps")
                for c in range(NQ7):
                    nc.tensor.matmul(ps[:, 512 * c:512 * c + 512],
                                     e_sb[:, 128 * c:128 * c + 128],
                                     src_ap, start=True, stop=True)
                tb = tablep.tile([128, H], F32, tag=tag)
                nc.scalar.copy(tb[:], ps[:])
                return tb

            def spmm(l, table, tag):
                """Returns (S1, S2) [128, 512] tiles of group sums."""
                s1 = smallp.tile([128, ROWS_PER_CORE], F32, tag=f"s1{tag}")
                s2 = smallp.tile([128, ROWS_PER_CORE], F32, tag=f"s2{tag}")
                for q in range(CHUNKS):
                    g = gpool.tile([128, IDX_PER_CHUNK], F32, tag="g")
                    nc.gpsimd.ap_gather(
                        g[:], table[:],
                        gidx_sb[l][:, 512 * q:512 * q + 512],
                        channels=128, num_elems=H, d=1,
                        num_idxs=IDX_PER_CHUNK)
                    g3 = g[:].rearrange("p (r k) -> p r k", k=K)
                    cols = slice(ROWS_PER_CHUNK * q,
                                 ROWS_PER_CHUNK * (q + 1))
                    nc.vector.tensor_reduce(
                        s1[:, cols], g3[:, :, 0:k1],
                        axis=mybir.AxisListType.X, op=mybir.AluOpType.add)
                    nc.vector.tensor_reduce(
                        s2[:, cols], g3[:, :, k1:K],
                        axis=mybir.AxisListType.X, op=mybir.AluOpType.add)
                return s1, s2

            h0_prev = None
            h1_prev = statep.tile([128, ROWS_PER_CORE], F32, tag="h1")
            nc.vector.memset(h1_prev[:], 0.0)

            for t in range(T):
                xt = xtp.tile([128, ROWS_PER_CORE], F32, tag="xt")
                nc.sync.dma_start(xt[:], xe_rep[t])

                h0_new = statep.tile([128, ROWS_PER_CORE], F32, tag="h0")
                if t == 0:
                    nc.scalar.activation(h0_new[:], xt[:],
                                         mybir.ActivationFunctionType.Relu)
                else:
                    tb0 = build_table(h0_prev[:], "t0")
                    s1, s2 = spmm(0, tb0, "a")
                    c1, c2 = cc[0]
                    u = smallp.tile([128, ROWS_PER_CORE], F32, tag="u0")
                    nc.vector.scalar_tensor_tensor(
                        u[:], s2[:], c2 / c1, s1[:],
                        mybir.AluOpType.mult, mybir.AluOpType.add)
                    u2 = smallp.tile([128, ROWS_PER_CORE], F32, tag="u0b")
                    nc.vector.scalar_tensor_tensor(
                        u2[:], u[:], c1, xt[:],
                        mybir.AluOpType.mult, mybir.AluOpType.add)
                    nc.scalar.activation(h0_new[:], u2[:],
                                         mybir.ActivationFunctionType.Relu)

                sum01 = statep.tile([128, ROWS_PER_CORE], F32, tag="s01")
                nc.vector.tensor_add(sum01[:], h0_new[:], h1_prev[:])

                tb1 = build_table(sum01[:], "t1")
                s1, s2 = spmm(1, tb1, "b")
                c1, c2 = cc[1]
                h1_new = statep.tile([128, ROWS_PER_CORE], F32, tag="h1")
                u = smallp.tile([128, ROWS_PER_CORE], F32, tag="u1")
                nc.vector.scalar_tensor_tensor(
                    u[:], s2[:], c2 / c1, s1[:],
                    mybir.AluOpType.mult, mybir.AluOpType.add)
                nc.scalar.activation(h1_new[:], u[:],
                                     mybir.ActivationFunctionType.Relu,
                                     scale=c1)

                nc.sync.dma_start(out_seq[t], h1_new[:])
                h0_prev, h1_prev = h0_new, h1_new

            nc.sync.dma_start(hfin[0], h0_prev[:])
            nc.sync.dma_start(hfin[1], h1_prev[:])

            for cm in reversed(rec_pool_cms):
                cm.__exit__(None, None, None)

    nc.compile()
    return nc


def _host_prep(x, w_ih, hh_idx, hh_vals):
    """Build per-core input maps + the constants that shape the program."""
    x = np.asarray(x, dtype=np.float32)
    w_ih = np.asarray(w_ih, dtype=np.float32)
    hh_idx = np.asarray(hh_idx)
    hh_vals = np.asarray(hh_vals, dtype=np.float32)

    # vals must be row-independent and piecewise-constant in k (2 groups)
    c_consts = []
    k1s = []
    for l in range(L):
        v0 = hh_vals[l, 0]
        assert np.all(hh_vals[l] == v0[None, :]), \
            "hh_vals must be row-independent"
        change = np.nonzero(v0 != v0[0])[0]
        if len(change) == 0:
            k1 = K - 8
            c1 = c2 = float(v0[0])
        else:
            k1 = int(change[0])
            c1, c2 = float(v0[0]), float(v0[k1])
            assert np.all(v0[:k1] == c1) and np.all(v0[k1:] == c2), \
                "hh_vals must have exactly two constant k-groups"
        c_consts.append((c1, c2))
        k1s.append(k1)
    assert k1s[0] == k1s[1], "same split expected per layer"
    k1 = k1s[0]

    # selection matrices: E_c[src_p, dst_p] = 1 iff src_p//16==c and
    # src_p%16 == dst_p%16
    esel = np.zeros((NQ7, 128, 128), np.float32)
    for c in range(NQ7):
        for lane in range(16):
            esel[c, 16 * c + lane, lane::16] = 1.0

    # gather index streams, wrapped per Q7 core
    gidx = np.zeros((L, 128, IDXCOLS), np.int16)
    for l in range(L):
        for c in range(NQ7):
            stream = hh_idx[l, ROWS_PER_CORE * c:
                            ROWS_PER_CORE * (c + 1), :].reshape(-1)
            gidx[l, 16 * c:16 * c + 16, :] = \
                stream.astype(np.int16).reshape(IDXCOLS, 16).T

    in_maps = []
    for dev in range(NCORES):
        xl = x[BL * dev:BL * (dev + 1)]            # [4, T, IN]
        xTl = xl.reshape(BL * T, IN).T.copy()      # [IN, BL*T]
        in_maps.append({
            "xT": np.ascontiguousarray(xTl),
            "wT": np.ascontiguousarray(w_ih.T),
            "esel": esel,
            "gidx": gidx,
        })
    return tuple(c_consts), k1, in_maps


def _assemble(results):
    out = np.zeros((B, T, H), np.float32)
    h_final = np.zeros((L, B, H), np.float32)
    for dev in range(NCORES):
        o = results[dev]["out_seq"].reshape(T, NQ7, 16, ROWS_PER_CORE)
        # out[4*dev+b, t, 512c+j'] = o[t, c, b, j']   (replica r=0)
        out[BL * dev:BL * (dev + 1)] = \
            o[:, :, :BL, :].transpose(2, 0, 1, 3).reshape(BL, T, H)
        hf = results[dev]["hfin"].reshape(L, NQ7, 16, ROWS_PER_CORE)
        h_final[:, BL * dev:BL * (dev + 1)] = \
            hf[:, :, :BL, :].transpose(0, 2, 1, 3).reshape(L, BL, H)
    return out, h_final


def kernel(x, w_ih, hh_idx, hh_vals):
    c_consts, k1, in_maps = _host_prep(x, w_ih, hh_idx, hh_vals)
    key = (c_consts, k1)
    if key not in _CACHE:
        _CACHE[key] = _build_program(c_consts, k1)
    nc = _CACHE[key]
    res = run_bass_kernel_spmd(nc, in_maps, list(range(NCORES)))
    return _assemble(res.results)


# revision 16
# speedup vs baseline: 5.7598x; 5.7598x over previous
"""Trainium2 Bass kernel for nn_BalRNN (2-layer sparse RNN).

Strategy (8 NeuronCores):
  - Grid: 2 batch-groups (16 batches each) x 4 row-quarters (1024 of the
    4096 hidden rows each).  NeuronCore dev = (dev//4 = batch group,
    dev%4 = row quarter).  The scan over T stays local; each timestep the
    row quarters of a batch group exchange their updated hidden-state
    slices with one AllGather (replica groups [[0..3],[4..7]]).
  - The sparse recurrent matmul out[b,i] = sum_k vals[i,k]*h[b, idx[i,k]]
    runs as ONE GPSIMD ap_gather per timestep: the index stream of layer 1
    at step t and layer 0 at step t+1 are merged (both only need h0(t) and
    h1(t-1), which the AllGather provides as a concatenated two-section
    table).  Each Q7 core's 16 partitions hold the tables of the 16
    batches, so every gathered 64-byte column is fully useful.
  - hh_vals is piecewise-constant in k (two groups), so the multiply
    folds into scalar constants applied after DVE segmented reduces.
  - The input projection xe = x @ w_ih.T (only this core's row quarter)
    runs once on TensorE at setup, staged to DRAM in gather layout.

Per-NeuronCore layout (128 SBUF partitions): partition p = 16*c + b,
c = Q7 core 0..7, b = local batch 0..15.  Q7 core c computes output rows
[my_quarter + 128c, my_quarter + 128c + 128).
"""

import sys

sys.path.insert(0, "/opt/trn_rl_repo")

import numpy as np

import concourse.bacc as bacc
import concourse.mybir as mybir
import concourse.tile as tile
from concourse.bass_utils import run_bass_kernel_spmd

B, T, IN = 32, 64, 512
L, H, K = 2, 4096, 64
NCORES = 8
NGROUP = 2                 # batch groups
NQUART = 4                 # row quarters
BL = B // NGROUP           # 16 batches per NeuronCore
NQ7 = 8
MYROWS = H // NQUART       # 1024 rows per NeuronCore
RPC = MYROWS // NQ7        # 128 rows per Q7 core per layer-section
NI = 2 * RPC * K           # 16384 merged indices per Q7 core per step
IDXCOLS = NI // 16         # 1024 idx columns per partition
F32 = mybir.dt.float32
I16 = mybir.dt.int16
GROUPS = [[0, 1, 2, 3], [4, 5, 6, 7]]

_CACHE = {}


def _build_program(c_consts, k1, variant=()):
    """c_consts: ((c1_l0,c2_l0),(c1_l1,c2_l1)); k1: k-split point."""
    nc = bacc.Bacc("TRN2", target_bir_lowering=False, debug=False,
                   num_devices=NCORES)

    xT = nc.dram_tensor("xT", [IN, BL * T], F32, kind="ExternalInput").ap()
    wT = nc.dram_tensor("wT", [IN, MYROWS], F32, kind="ExternalInput").ap()
    gidx = nc.dram_tensor("gidx", [128, IDXCOLS], I16,
                          kind="ExternalInput").ap()
    out_seq = nc.dram_tensor("out_seq", [T, 128, RPC], F32,
                             kind="ExternalOutput").ap()
    hfin = nc.dram_tensor("hfin", [L, 128, RPC], F32,
                          kind="ExternalOutput").ap()
    xe_rep = nc.dram_tensor("xe_rep", [T, 128, RPC], F32).ap()

    (c1_0, c2_0), (c1_1, c2_1) = c_consts

    with tile.TileContext(nc) as tc:
        with (
            tc.tile_pool(name="const", bufs=1) as constp,
            tc.tile_pool(name="ps", bufs=1, space="PSUM") as psp,
            tc.tile_pool(name="dram", bufs=2, space="DRAM") as dramp,
        ):
            gidx_sb = constp.tile([128, IDXCOLS], I16)
            nc.sync.dma_start(gidx_sb[:], gidx[:])

            # ---- xe = x_group @ w_quarter.T (once, on PE) ----
            setupp_cm = tc.tile_pool(name="setup", bufs=1)
            setupp = setupp_cm.__enter__()
            M = BL * T                                  # 1024
            xT_sb = setupp.tile([128, 4 * M], F32)      # [j, (jb, m)]
            nc.sync.dma_start(
                xT_sb[:].rearrange("p (jb m) -> p jb m", jb=4),
                xT.rearrange("(jb p) m -> p jb m", p=128))
            wT_sb = setupp.tile([128, 4 * MYROWS], F32)  # [j, (jb, i)]
            nc.sync.dma_start(
                wT_sb[:].rearrange("p (jb i) -> p jb i", jb=4),
                wT.rearrange("(jb p) i -> p jb i", p=128))

            for mb in range(M // 128):
                ps = psp.tile([128, MYROWS], F32, tag="xps")
                for nch in range(MYROWS // 512):
                    for jb in range(4):
                        nc.tensor.matmul(
                            ps[:, 512 * nch:512 * nch + 512],
                            xT_sb[:, M * jb + 128 * mb:
                                  M * jb + 128 * mb + 128],
                            wT_sb[:, MYROWS * jb + 512 * nch:
                                  MYROWS * jb + 512 * nch + 512],
                            start=(jb == 0), stop=(jb == 3))
                xe_sb = setupp.tile([128, MYROWS], F32, tag="xe_sb")
                nc.scalar.copy(xe_sb[:], ps[:])
                # xe_rep[t, 16c+b, r] = xe[(b,t), 128c + r]
                # partition m of chunk mb: b = (128*mb + m)//T, t = m%T
                # (T=64 -> two local batches per chunk)
                for b_loc in range(128 // T):
                    b = (128 * mb) // T + b_loc
                    src = xe_sb[T * b_loc:T * b_loc + T, :]
                    dst = xe_rep.rearrange(
                        "t (c l) j -> t c l j", l=16)[:, :, b, :]
                    nc.sync.dma_start(dst, src.rearrange(
                        "t (c j) -> t c j", c=NQ7))
            setupp_cm.__exit__(None, None, None)

            # ---- recurrence ----
            rec_pool_cms = (
                tc.tile_pool(name="table", bufs=1),
                tc.tile_pool(name="gbuf", bufs=2),
                tc.tile_pool(name="state", bufs=3),
                tc.tile_pool(name="small", bufs=3),
                tc.tile_pool(name="xt", bufs=3),
            )
            tablep, gpool, statep, smallp, xtp = \
                (cm.__enter__() for cm in rec_pool_cms)

            xt0 = xtp.tile([128, RPC], F32, tag="xt")
            nc.sync.dma_start(xt0[:], xe_rep[0])
            h0 = statep.tile([128, RPC], F32, tag="h0")
            nc.scalar.activation(h0[:], xt0[:],
                                 mybir.ActivationFunctionType.Relu)
            h1_prev = statep.tile([128, RPC], F32, tag="h1")
            nc.vector.memset(h1_prev[:], 0.0)
            sum01 = statep.tile([128, RPC], F32, tag="s01")
            nc.vector.tensor_add(sum01[:], h0[:], h1_prev[:])

            for t in range(T):
                # AllGather [sum01(t) | h0(t)] within the 4 row quarters
                agin = dramp.tile([2, 128, RPC], F32, tag="agin")
                nc.sync.dma_start(agin[0], sum01[:])
                nc.sync.dma_start(agin[1], h0[:])
                agout = dramp.tile([NQUART, 2, 128, RPC], F32, tag="agout")
                if "nocc" in variant:
                    nc.sync.dma_start(agout[0], agin[:])
                else:
                    nc.gpsimd.collective_compute(
                        "AllGather", mybir.AluOpType.bypass,
                        replica_groups=GROUPS,
                        ins=[agin.opt()], outs=[agout.opt()])

                # table_cat[16c+b, 4096*s + 1024*g + 128*c' + r]
                #   = agout[g, s, 16c'+b, r]
                # table column layout == agout walk order per batch lane:
                # tb[16c+b, ((g*2+s)*8+cc)*128 + r] = agout[g, s, 16cc+b, r]
                # (the host encodes gather indices in this layout)
                tb = tablep.tile([128, 2 * H], F32, tag="tb")
                agv = agout[:].rearrange(
                    "g s (cc l) r -> l (g s) cc r", l=16)
                for c in range(NQ7):
                    nc.sync.dma_start(tb[16 * c:16 * c + 16, :], agv)

                g = gpool.tile([128, NI], F32, tag="g")
                if "nogather" in variant:
                    nc.vector.memset(g[:, 0:4], 0.0)
                else:
                    nc.gpsimd.ap_gather(
                        g[:], tb[:], gidx_sb[:], channels=128,
                        num_elems=2 * H, d=1, num_idxs=NI)

                g3 = g[:].rearrange("p (r k) -> p r k", k=K)
                s1 = smallp.tile([128, 2 * RPC], F32, tag="s1")
                s2 = smallp.tile([128, 2 * RPC], F32, tag="s2")
                nc.vector.tensor_reduce(
                    s1[:], g3[:, :, 0:k1],
                    axis=mybir.AxisListType.X, op=mybir.AluOpType.add)
                nc.vector.tensor_reduce(
                    s2[:], g3[:, :, k1:K],
                    axis=mybir.AxisListType.X, op=mybir.AluOpType.add)

                # layer-1 result -> h1(t)
                u_a = smallp.tile([128, RPC], F32, tag="ua")
                nc.vector.scalar_tensor_tensor(
                    u_a[:], s2[:, 0:RPC], c2_1 / c1_1, s1[:, 0:RPC],
                    mybir.AluOpType.mult, mybir.AluOpType.add)
                h1 = statep.tile([128, RPC], F32, tag="h1")
                nc.scalar.activation(h1[:], u_a[:],
                                     mybir.ActivationFunctionType.Relu,
                                     scale=c1_1)
                nc.sync.dma_start(out_seq[t], h1[:])

                if t == T - 1:
                    nc.sync.dma_start(hfin[0], h0[:])
                    nc.sync.dma_start(hfin[1], h1[:])
                    break

                # layer-0 result -> h0(t+1)
                xt = xtp.tile([128, RPC], F32, tag="xt")
                nc.sync.dma_start(xt[:], xe_rep[t + 1])
                u_b = smallp.tile([128, RPC], F32, tag="ub")
                nc.vector.scalar_tensor_tensor(
                    u_b[:], s2[:, RPC:2 * RPC], c2_0 / c1_0,
                    s1[:, RPC:2 * RPC],
                    mybir.AluOpType.mult, mybir.AluOpType.add)
                u_c = smallp.tile([128, RPC], F32, tag="uc")
                nc.vector.scalar_tensor_tensor(
                    u_c[:], u_b[:], c1_0, xt[:],
                    mybir.AluOpType.mult, mybir.AluOpType.add)
                h0 = statep.tile([128, RPC], F32, tag="h0")
                nc.scalar.activation(h0[:], u_c[:],
                                     mybir.ActivationFunctionType.Relu)
                sum01 = statep.tile([128, RPC], F32, tag="s01")
                nc.vector.tensor_add(sum01[:], h0[:], h1[:])

            for cm in reversed(rec_pool_cms):
                cm.__exit__(None, None, None)

    nc.compile()
    return nc


def _host_prep(x, w_ih, hh_idx, hh_vals):
    x = np.asarray(x, dtype=np.float32)
    w_ih = np.asarray(w_ih, dtype=np.float32)
    hh_idx = np.asarray(hh_idx)
    hh_vals = np.asarray(hh_vals, dtype=np.float32)

    c_consts = []
    k1s = []
    for l in range(L):
        v0 = hh_vals[l, 0]
        assert np.all(hh_vals[l] == v0[None, :]), \
            "hh_vals must be row-independent"
        change = np.nonzero(v0 != v0[0])[0]
        if len(change) == 0:
            k1 = K - 8
            c1 = c2 = float(v0[0])
        else:
            k1 = int(change[0])
            c1, c2 = float(v0[0]), float(v0[k1])
            assert np.all(v0[:k1] == c1) and np.all(v0[k1:] == c2), \
                "hh_vals must have exactly two constant k-groups"
        c_consts.append((c1, c2))
        k1s.append(k1)
    assert k1s[0] == k1s[1], "same split expected per layer"
    k1 = k1s[0]

    in_maps = []
    for dev in range(NCORES):
        bg, rq = dev // NQUART, dev % NQUART
        rows0 = MYROWS * rq
        # merged index stream per Q7 core: layer-1 rows then layer-0 rows
        # (layer-0 table section offset +H)
        # table offset of h-section s, global row j:
        #   (j//1024)*2048 + s*1024 + (j % 1024)
        def off(j, s):
            return (j // 1024) * 2048 + s * 1024 + (j % 1024)

        gidx_dev = np.zeros((128, IDXCOLS), np.int16)
        for c in range(NQ7):
            rows = slice(rows0 + RPC * c, rows0 + RPC * (c + 1))
            stream = np.concatenate([
                off(hh_idx[1, rows, :].reshape(-1), 0),
                off(hh_idx[0, rows, :].reshape(-1), 1),
            ]).astype(np.int16)
            gidx_dev[16 * c:16 * c + 16, :] = stream.reshape(IDXCOLS, 16).T

        xl = x[BL * bg:BL * (bg + 1)]               # [16, T, IN]
        in_maps.append({
            "xT": np.ascontiguousarray(xl.reshape(BL * T, IN).T),
            "wT": np.ascontiguousarray(
                w_ih[rows0:rows0 + MYROWS, :].T),
            "gidx": gidx_dev,
        })
    return tuple(c_consts), k1, in_maps


def _assemble(results):
    out = np.zeros((B, T, H), np.float32)
    h_final = np.zeros((L, B, H), np.float32)
    for dev in range(NCORES):
        bg, rq = dev // NQUART, dev % NQUART
        o = results[dev]["out_seq"].reshape(T, NQ7, 16, RPC)
        out[BL * bg:BL * (bg + 1), :, MYROWS * rq:MYROWS * (rq + 1)] = \
            o.transpose(2, 0, 1, 3).reshape(16, T, MYROWS)
        hf = results[dev]["hfin"].reshape(L, NQ7, 16, RPC)
        h_final[:, BL * bg:BL * (bg + 1),
                MYROWS * rq:MYROWS * (rq + 1)] = \
            hf.transpose(0, 2, 1, 3).reshape(L, 16, MYROWS)
    return out, h_final


def kernel(x, w_ih, hh_idx, hh_vals):
    c_consts, k1, in_maps = _host_prep(x, w_ih, hh_idx, hh_vals)
    key = (c_consts, k1)
    if key not in _CACHE:
        _CACHE[key] = _build_program(c_consts, k1)
    nc = _CACHE[key]
    res = run_bass_kernel_spmd(nc, in_maps, list(range(NCORES)))
    return _assemble(res.results)
